# revision 45
# baseline (speedup 1.0000x reference)
"""Grouped GEMM (MoE routing) Trainium2 kernel.

Expert-parallel across 8 NeuronCores with size-sorted slot assignment:
experts are sorted by token count and slot s on every core holds the
experts of size-rank [8s, 8s+8), so one SPMD program with per-slot
capacities cap_s = roundup4(max count in rank group) serves all cores.

Flipped matmul orientation: weights are the stationary operand
([128 K, 128 DOUT] fp8 e3m4 blocks, per-expert scale undone on host)
and token tiles stream as the moving operand ([128 K, cap] bf16), so
PE stream time is proportional to actual routed tokens instead of
ceil(cap/128) full 128-lane tiles. Output leaves transposed
([13, 128, sumcap] bf16) and is unpacked on host.
"""
import ml_dtypes
import numpy as np

import concourse.bass as bass
import concourse.mybir as mybir
import concourse.tile as tile
from concourse import bacc
from concourse.bass_utils import run_bass_kernel_spmd

G, T, DIN, DOUT = 64, 8192, 2560, 1664
NCORES = 8
EPC = G // NCORES   # expert slots per core
KC = DIN // 128     # 20 contraction chunks
OC = DOUT // 128    # 13 output-row chunks
W8MAX = 15.0        # e3m4 scale target (max normal 15.5)

_cache = {}


def _build(caps):
    offs = np.concatenate([[0], np.cumsum(caps)]).astype(int)
    sumcap = int(offs[-1])
    nc = bacc.Bacc(trn_type="TRN2", debug=False)
    bf16 = mybir.dt.bfloat16
    e3 = mybir.dt.float8e3
    xt = nc.dram_tensor("xt", [128, KC * sumcap], bf16, kind="ExternalInput").ap()
    w = nc.dram_tensor("w", [EPC, KC, 128, DOUT], e3, kind="ExternalInput").ap()
    out = nc.dram_tensor(
        "out", [OC, 128, sumcap], bf16, kind="ExternalOutput"
    ).ap()
    with tile.TileContext(nc) as tc:
        with (
            tc.tile_pool(name="xtp", bufs=3) as xt_pool,
            tc.tile_pool(name="xt0p", bufs=2 * KC) as xt0_pool,
            tc.tile_pool(name="wp", bufs=72) as w_pool,
            tc.tile_pool(name="op", bufs=16) as o_pool,
            tc.tile_pool(name="ps", bufs=1, space="PSUM") as ps_pool,
        ):
            # PE warmup: dummy matmuls during the initial DMA fill keep the
            # HAM clock-gate at 8/8 so slot 0's real matmuls run at 2.4 GHz
            wm_x = xt_pool.tile([128, 512], bf16, tag="wm", name="warm_x")
            nc.vector.memset(wm_x[:], 0)
            wm_ps = ps_pool.tile([128, 512], mybir.dt.float32, tag="wm",
                                 name="warm_ps")
            for i in range(20):
                nc.tensor.matmul(wm_ps[:], wm_x[:, :128], wm_x[:],
                                 start=True, stop=True)
            # slots 0-1: per-k token tiles, issued before any out-DMAs
            # exist on the gpsimd queue (in-order queue: a waiting out-DMA
            # would block them); also lets the first matmul start as soon
            # as one 512B/partition chunk lands
            xk = {}
            for s in range(min(2, EPC)):
                cap = int(caps[s])
                off = int(offs[s])
                for k in range(KC):
                    if cap == 0:
                        continue
                    xk[s, k] = xt0_pool.tile([128, cap], bf16, tag=f"xt0{s}",
                                             name=f"xt0_{s}_{k}")
                    nc.gpsimd.dma_start(
                        xk[s, k][:],
                        xt[:, KC * off + k * cap:KC * off + (k + 1) * cap],
                    )
            xts_pending = {}
            sched = list(range(EPC))
            for pos, s in enumerate(sched):
                cap = int(caps[s])
                if cap == 0:
                    continue
                off = int(offs[s])
                if s < 2:
                    xts = lambda k, s_=s: xk[s_, k][:]
                else:
                    # emitted on the sync ring in the previous slot's body
                    xts = xts_pending[s]
                # prefetch next slot's tokens on the sync ring, sequenced
                # after this point's w tiles so they never compete with the
                # in-progress fill (and never block: inputs have no deps)
                nxt = sched[pos + 1] if pos + 1 < len(sched) else -1
                if nxt >= 2 and int(caps[nxt]) > 0:
                    ncap = int(caps[nxt])
                    noff = int(offs[nxt])
                    nxt_sb = xt_pool.tile([128, KC * ncap], bf16, tag="xt",
                                          name=f"xt{nxt}")
                    nc.sync.dma_start(
                        nxt_sb[:], xt[:, KC * noff:KC * (noff + ncap)]
                    )
                    xts_pending[nxt] = \
                        lambda k, t=nxt_sb, c=ncap: t[:, k * c:(k + 1) * c]
                w_sb = {}
                for k in range(KC):
                    w_sb[k] = w_pool.tile([128, DOUT], e3, tag="w",
                                          name=f"w{s}_{k}")
                    if s == 0:
                        eng = nc.sync if k % 2 == 0 else nc.scalar
                    elif s == 1:
                        eng = nc.scalar if k % 2 == 0 else nc.sync
                    else:
                        eng = nc.sync if k % 2 == 0 else nc.scalar
                    eng.dma_start(w_sb[k][:], w[s, k])
                if s == 0:
                    # k-major: each w k-tile feeds all oc chains as it
                    # lands, so the PE keeps pace with the initial fill
                    for og in ((0, 7), (7, 13)):
                        psums = {}
                        for oc in range(*og):
                            psums[oc] = ps_pool.tile(
                                [128, cap], mybir.dt.float32,
                                tag=f"z{oc - og[0]}", name=f"psum_0_{oc}",
                            )
                        for k in range(KC):
                            for oc in range(*og):
                                nc.tensor.matmul(
                                    psums[oc][:],
                                    w_sb[k][:, oc * 128:oc * 128 + 128],
                                    xts(k),
                                    start=(k == 0),
                                    stop=(k == KC - 1),
                                )
                        for oc in range(*og):
                            o_sb = o_pool.tile([128, cap], bf16, tag="o",
                                               name=f"o_{s}_{oc}")
                            nc.vector.tensor_copy(o_sb[:], psums[oc][:])
                            nc.gpsimd.dma_start(out[oc, :, off:off + cap],
                                                o_sb[:])
                    continue
                oeng = (lambda oc: nc.gpsimd) if pos < EPC - 2 else \
                    (lambda oc: nc.sync if oc % 2 == 0 else nc.scalar)
                for oc in range(OC):
                    psum = ps_pool.tile(
                        [128, cap], mybir.dt.float32, tag=f"z{oc % 2}",
                        name=f"psum_{s}_{oc}",
                    )
                    for k in range(KC):
                        nc.tensor.matmul(
                            psum[:],
                            w_sb[k][:, oc * 128:oc * 128 + 128],
                            xts(k),
                            start=(k == 0),
                            stop=(k == KC - 1),
                        )
                    o_sb = o_pool.tile([128, cap], bf16, tag="o",
                                       name=f"o_{s}_{oc}")
                    nc.vector.tensor_copy(o_sb[:], psum[:])
                    oeng(oc).dma_start(out[oc, :, off:off + cap], o_sb[:])
    nc.compile()
    return nc


def _run(inputs, trace=False):
    x = np.asarray(inputs["input"], dtype=np.float32)
    w = np.ascontiguousarray(np.asarray(inputs["weight"], dtype=np.float32))
    counts = np.asarray(inputs["tokens_per_expert"], dtype=np.int64)
    starts = np.concatenate([[0], np.cumsum(counts)[:-1]])

    order = np.argsort(-counts, kind="stable")  # experts by size rank
    # slot s, core c -> expert order[s*NCORES + c]; capacity = rank-group max
    caps = tuple(
        int(np.ceil(max(1, counts[order[s * NCORES:(s + 1) * NCORES]].max()) / 4) * 4)
        for s in range(EPC)
    )
    offs = np.concatenate([[0], np.cumsum(caps)]).astype(int)
    sumcap = int(offs[-1])

    if caps not in _cache:
        _cache[caps] = _build(caps)
    nc = _cache[caps]

    # per-expert fp8 scale
    wmax = np.abs(w).max(axis=(1, 2))
    alpha = np.where(wmax > 0, W8MAX / np.maximum(wmax, 1e-30), 1.0)

    in_maps = []
    for c in range(NCORES):
        xt_pack = np.zeros((128, KC * sumcap), dtype=ml_dtypes.bfloat16)
        w_pack = np.empty((EPC, KC, 128, DOUT), dtype=ml_dtypes.float8_e3m4)
        for s in range(EPC):
            g = int(order[s * NCORES + c])
            cnt = int(counts[g])
            cap = caps[s]
            if cnt:
                # [cnt, DIN] -> [128, KC, cnt] (partition, k-chunk, token)
                xs = x[starts[g]:starts[g] + cnt].T.reshape(KC, 128, cnt)
                xt_pack[:, KC * offs[s]:KC * (offs[s] + cap)] \
                    .reshape(128, KC, cap)[:, :, :cnt] = xs.swapaxes(0, 1)
            w_pack[s] = (w[g] * alpha[g]).astype(ml_dtypes.float8_e3m4) \
                .reshape(KC, 128, DOUT)
        in_maps.append({"xt": xt_pack, "w": w_pack})

    kw = {"trace_cores": list(range(NCORES))} if trace else {}
    res = None
    for attempt in range(3):
        try:
            res = run_bass_kernel_spmd(nc, in_maps, core_ids=list(range(NCORES)),
                                       trace=trace, **kw)
            break
        except Exception:
            # transient NRT_EXEC_UNIT_UNRECOVERABLE on first execution of a
            # fresh NEFF has been observed; retry
            if attempt == 2:
                raise

    out = np.empty((T, DOUT), dtype=np.float32)
    for c in range(NCORES):
        for s in range(EPC):
            g = int(order[s * NCORES + c])
            cnt = int(counts[g])
            if cnt:
                # [OC, 128, cnt] -> [cnt, DOUT]
                y = res.results[c]["out"][:, :, offs[s]:offs[s] + cnt]
                out[starts[g]:starts[g] + cnt] = \
                    y.transpose(2, 0, 1).reshape(cnt, DOUT).astype(np.float32) \
                    * (1.0 / alpha[g])
    return out, res


def kernel(**inputs) -> np.ndarray:
    return _run(inputs)[0]
